# revision 10
# baseline (speedup 1.0000x reference)
"""Trainium2 Bass kernel for CustomBCEWithLogitsLoss (topk masking).

Math: with e = softplus(l) - l*t (elementwise BCE-with-logits),
  out = mean_all(e) + BCE_L * mean_{top20-by-logit per row}(e)
since top-k of sigmoid(logits) = top-k of logits, and the reference's
top-k BCE term equals e at those positions (-100 clamps never bind).

Strategy (vs a straight f32 port):
  * Host casts both inputs to bf16 - halves HBM traffic (20.5MB/core).
  * The whole top-k runs in the EXP DOMAIN: V = exp(l) (bf16, computed
    by ACT for softplus anyway) is monotone in l, so per-chunk max8,
    the top-20 cascade, tau, and the (V >= tau) mask all use V. L is
    freed right after Exp + product.
  * GPSIMD computes LT = l*t (bf16) - the only bulk GPSIMD work.
  * TensorE (otherwise idle) computes sum_all(l*t) for the BCE term:
    ones[128,1]^T @ LT chunks accumulated into one PSUM bank across
    all tiles (partition-dim reduction), one ACT read at the end.
    This removes every row-sum accumulation pass for l*t.
  * DVE: 8x max8 over 1250-col chunks, top-24 cascade, and one
    full-row masked-accum STT per tile: MLT = (V >= tau)*LT in place.

Per-core engine budget (us): DVE ~100, GPSIMD ~80, ACT ~74, PE ~25,
DMA ~62.

Exactness: selection matches the reference's f32 top-20 exactly unless
bf16 quantization ties the 20/21 boundary (tau2 == tau in exact bf16
bits) or a chunk's top-8 may have missed a top-20 value (ch8 >= tau).
Flagged rows (~6%) are recomputed exactly on host from the original
f32 inputs. Host combines partials in f64.
"""

import numpy as np
import ml_dtypes

B, N, K = 4096, 10000, 20
NCORES = 8
R = B // NCORES          # rows per core
P = 128                  # partitions
NT = R // P              # tiles per core
H = N // 2               # half-row width
CCH = 8                  # candidate chunks per row
W = N // CCH             # candidate chunk width (1250)
MMW = 500                # matmul moving chunk width (PSUM bank fit)
SLOTS = 96               # per-tile output slots
ACT_TABLE = "natural_log_exp_and_others"

_PROGRAM = None


def _build_program():
    import concourse.bacc as bacc
    import concourse.tile as tile
    import concourse.mybir as mybir
    from concourse.hw_specs import get_activation_tables

    nc = bacc.Bacc("TRN2", target_bir_lowering=False, debug=False)
    f32 = mybir.dt.float32
    bf16 = mybir.dt.bfloat16
    logits = nc.dram_tensor("logits", [R, N], bf16, kind="ExternalInput")
    targets = nc.dram_tensor("targets", [R, N], bf16, kind="ExternalInput")
    out = nc.dram_tensor("partials", [P, NT * SLOTS], f32,
                         kind="ExternalOutput")
    Lr = logits.ap().rearrange("(t p) n -> t p n", p=P)
    Tr = targets.ap().rearrange("(t p) n -> t p n", p=P)

    AF = mybir.ActivationFunctionType
    OP = mybir.AluOpType
    NMM = N // MMW

    with tile.TileContext(nc) as tc:
        with (
            tc.tile_pool(name="pL", bufs=4) as pL,
            tc.tile_pool(name="pT", bufs=4) as pT,
            tc.tile_pool(name="pLT", bufs=2) as pLT,
            tc.tile_pool(name="pV", bufs=2) as pV,
            tc.tile_pool(name="pScr", bufs=1) as pScr,
            tc.tile_pool(name="cnd", bufs=2) as cnd,
            tc.tile_pool(name="small", bufs=2) as small,
            tc.tile_pool(name="one", bufs=1) as one,
            tc.tile_pool(name="pJ", bufs=1) as pJ,
            tc.tile_pool(name="outp", bufs=1) as outp,
            tc.tile_pool(name="psum", bufs=1, space="PSUM") as psum,
        ):
            OUT = outp.tile([P, NT * SLOTS], f32)
            nc.gpsimd.memset(OUT, 0.0)
            ones = one.tile([P, 1], bf16)
            nc.gpsimd.memset(ones, 1.0)
            ltacc = psum.tile([1, MMW], f32)
            junk = pJ.tile([P, N], bf16)
            pend = None   # (Vt, LTt, mall, tau, s0) of the previous tile

            def emit_pend(Vt, LTt, cand, mall, tau, s0):
                # masked sum: MLT = (V >= tau) * LT, full row (result
                # discarded to a junk tile; only the accum matters)
                nc.vector.scalar_tensor_tensor(
                    out=junk, in0=Vt, scalar=tau, in1=LTt,
                    op0=OP.is_ge, op1=OP.mult,
                    accum_out=OUT[:, s0 + 2:s0 + 3])
                # sum_top softplus from the top-20 exp values: ln(V + 1)
                x20 = small.tile([P, 20], bf16, tag="x20")
                nc.scalar.activation(x20, mall[:, 0:20], AF.Ln,
                                     bias=1.0, scale=1.0,
                                     accum_out=OUT[:, s0 + 3:s0 + 4])
                # exactness channels on ACT (DVE copies stall against
                # concurrent GPSIMD SBUF traffic). cand is dumped after
                # match_replace: -1 entries mean "was in the top-16",
                # which the host flags conservatively.
                nc.scalar.activation(OUT[:, s0 + 8:s0 + 8 + CCH * 8],
                                     cand, AF.Copy)
                nc.scalar.activation(OUT[:, s0 + 72:s0 + 96],
                                     mall, AF.Copy)

            for t in range(NT):
                s0 = t * SLOTS
                LTt = pLT.tile([P, N], bf16, tag="LT")
                Vt = pV.tile([P, N], bf16, tag="V")
                for h in range(2):
                    sl = slice(h * H, (h + 1) * H)
                    Lh = pL.tile([P, H], bf16, tag="L")
                    nc.sync.dma_start(Lh, Lr[t][:, sl])
                    Th = pT.tile([P, H], bf16, tag="T")
                    nc.sync.dma_start(Th, Tr[t][:, sl])
                    nc.gpsimd.tensor_tensor(
                        out=LTt[:, sl], in0=Lh, in1=Th, op=OP.mult)
                    # V = exp(l); softplus accum via Ln(V + 1)
                    nc.scalar.activation(Vt[:, sl], Lh, AF.Exp)
                    scr = pScr.tile([P, H], bf16, tag="scr")
                    nc.scalar.activation(scr, Vt[:, sl], AF.Ln,
                                         bias=1.0, scale=1.0,
                                         accum_out=OUT[:, s0 + h:s0 + h + 1])
                    # sum_all(l*t): TensorE partition-sum of LT chunks
                    # into the persistent PSUM accumulator
                    for c in range(NMM // 2):
                        c0 = h * H + c * MMW
                        nc.tensor.matmul(
                            ltacc, ones, LTt[:, c0:c0 + MMW],
                            start=(t == 0 and h == 0 and c == 0),
                            stop=(t == NT - 1 and h == 1
                                  and c == NMM // 2 - 1))

                # top-20 in exp domain: per-chunk top-8, then cascade
                cand = cnd.tile([P, CCH * 8], bf16, tag="cand")
                for c in range(CCH):
                    nc.vector.max(out=cand[:, c * 8:(c + 1) * 8],
                                  in_=Vt[:, c * W:(c + 1) * W])
                # deferred masked sum + top-20 softplus of the previous
                # tile, emitted after max8 so the mask STT does not
                # overlap GPSIMD's products (SBUF port interference)
                if pend is not None:
                    emit_pend(*pend)
                mall = small.tile([P, 24], bf16, tag="mall")
                nc.vector.max(out=mall[:, 0:8], in_=cand)
                nc.vector.match_replace(out=cand, in_to_replace=mall[:, 0:8],
                                        in_values=cand, imm_value=-1.0)
                nc.vector.max(out=mall[:, 8:16], in_=cand)
                nc.vector.match_replace(out=cand, in_to_replace=mall[:, 8:16],
                                        in_values=cand, imm_value=-1.0)
                nc.vector.max(out=mall[:, 16:24], in_=cand)
                tau = mall[:, 19:20]   # 20th largest V; mall[:, 20] = 21st

                pend = (Vt, LTt, cand, mall, tau, s0)

            emit_pend(*pend)
            # read out the PSUM l*t total (one scalar on partition 0)
            scr5 = small.tile([1, MMW], f32, tag="psread")
            nc.scalar.activation(scr5, ltacc, AF.Copy,
                                 accum_out=OUT[0:1, 4:5])
            nc.sync.dma_start(out.ap(), OUT)

    # Force every activation onto one table (Exp+Ln+Copy live together
    # in natural_log_exp_and_others) so the engine never reloads tables.
    tabs = get_activation_tables(nc.m.arch)
    saved = {k: set(v) for k, v in tabs.items()}
    try:
        for k in tabs:
            if k != ACT_TABLE:
                tabs[k] = set()
        nc.compile()
    finally:
        for k, v in saved.items():
            tabs[k] = v
    return nc


def _get_program():
    global _PROGRAM
    if _PROGRAM is None:
        _PROGRAM = _build_program()
    return _PROGRAM


def _run_on_cores(logits, targets, trace=False, **kw):
    from concourse import bass_utils
    nc = _get_program()
    bf = ml_dtypes.bfloat16
    Lb = np.asarray(logits, dtype=np.float32).astype(bf)
    Tb = np.asarray(targets, dtype=np.float32).astype(bf)
    in_maps = [
        {"logits": np.ascontiguousarray(Lb[c * R:(c + 1) * R]),
         "targets": np.ascontiguousarray(Tb[c * R:(c + 1) * R])}
        for c in range(NCORES)
    ]
    return bass_utils.run_bass_kernel_spmd(
        nc, in_maps, core_ids=list(range(NCORES)), trace=trace, **kw)


def _host_fix_rows(logits, targets, rows):
    """Exact per-row recompute of the top-20 term, replicating the
    reference's tie-breaking (top_k on f32 sigmoid, stable by index)."""
    out = {}
    for r in rows:
        l = logits[r].astype(np.float32)
        t = targets[r].astype(np.float64)
        p = (1.0 / (1.0 + np.exp(-l.astype(np.float64)))).astype(np.float32)
        idx = np.argsort(-p, kind="stable")[:K]
        ld = l[idx].astype(np.float64)
        td = t[idx]
        sp = np.maximum(ld, 0) + np.log1p(np.exp(-np.abs(ld)))
        out[r] = float(np.sum(sp - ld * td))
    return out


def kernel(logits, targets, BCE_L):
    logits = np.asarray(logits, dtype=np.float32)
    targets = np.asarray(targets, dtype=np.float32)
    res = _run_on_cores(logits, targets)
    # partials[core]: [P, NT*SLOTS]; global row = core*R + t*P + p
    # slots: 0-1 sum softplus halves; 2 masked l*t; 3 sum_top softplus;
    #        8..72 candidate dump; 72..96 top-24 dump (exp domain);
    #        tile0 slot 4 partition 0 = core-wide sum l*t (from PSUM)
    bce_sum = 0.0
    me = np.zeros((NCORES, NT, P), dtype=np.float64)
    flag = np.zeros((NCORES, NT, P), dtype=bool)
    for c in range(NCORES):
        par = res.results[c]["partials"].astype(np.float64)
        bce_sum -= float(par[0, 4])
        for t in range(NT):
            s0 = t * SLOTS
            bce_sum += float(np.sum(par[:, s0:s0 + 2]))
            me[c, t] = par[:, s0 + 3] - par[:, s0 + 2]
            tau = par[:, s0 + 72 + 19]
            tau2 = par[:, s0 + 72 + 20]
            ch8 = par[:, s0 + 8:s0 + 8 + CCH * 8].reshape(P, CCH, 8)
            ch8 = ch8[:, :, 7]
            replaced = (ch8 == -1.0).any(axis=1)
            flag[c, t] = (ch8.max(axis=1) >= tau) | replaced | (tau2 == tau)
    me_rows = me.reshape(-1)
    bad = np.nonzero(flag.reshape(-1))[0]
    if bad.size:
        fixes = _host_fix_rows(logits, targets, bad.tolist())
        for r, v in fixes.items():
            me_rows[r] = v
    out = bce_sum / (B * N) + float(BCE_L[0]) * float(me_rows.sum()) / (B * K)
    return np.array(out, dtype=np.float32)


# revision 11
# speedup vs baseline: 1.0382x; 1.0382x over previous
"""Trainium2 Bass kernel for CustomBCEWithLogitsLoss (topk masking).

Math: with e = softplus(l) - l*t (elementwise BCE-with-logits),
  out = mean_all(e) + BCE_L * mean_{top20-by-logit per row}(e)
since top-k of sigmoid(logits) = top-k of logits, and the reference's
top-k BCE term equals e at those positions (-100 clamps never bind).

Strategy (vs a straight f32 port):
  * Host casts both inputs to bf16 - halves HBM traffic (20.5MB/core).
  * The whole top-k runs in the EXP DOMAIN: V = exp(l) (bf16, computed
    by ACT for softplus anyway) is monotone in l, so per-chunk max8,
    the top-20 cascade, tau, and the (V >= tau) mask all use V. L is
    freed right after Exp + product.
  * GPSIMD computes LT = l*t (bf16) - the only bulk GPSIMD work.
  * TensorE (otherwise idle) computes sum_all(l*t) for the BCE term:
    ones[128,1]^T @ LT chunks accumulated into one PSUM bank across
    all tiles (partition-dim reduction), one ACT read at the end.
    This removes every row-sum accumulation pass for l*t.
  * DVE: 8x max8 over 1250-col chunks, top-24 cascade, and one
    full-row masked-accum STT per tile: MLT = (V >= tau)*LT in place.

Per-core engine budget (us): DVE ~100, GPSIMD ~80, ACT ~74, PE ~25,
DMA ~62.

Exactness: selection matches the reference's f32 top-20 exactly unless
bf16 quantization ties the 20/21 boundary (tau2 == tau in exact bf16
bits) or a chunk's top-8 may have missed a top-20 value (ch8 >= tau).
Flagged rows (~6%) are recomputed exactly on host from the original
f32 inputs. Host combines partials in f64.
"""

import numpy as np
import ml_dtypes

B, N, K = 4096, 10000, 20
NCORES = 8
R = B // NCORES          # rows per core
P = 128                  # partitions
NT = R // P              # tiles per core
H = N // 2               # half-row width
CCH = 8                  # candidate chunks per row
W = N // CCH             # candidate chunk width (1250)
MMW = 500                # matmul moving chunk width (PSUM bank fit)
SLOTS = 96               # per-tile output slots
ACT_TABLE = "natural_log_exp_and_others"

_PROGRAM = None


def _build_program():
    import concourse.bacc as bacc
    import concourse.tile as tile
    import concourse.mybir as mybir
    from concourse.hw_specs import get_activation_tables

    nc = bacc.Bacc("TRN2", target_bir_lowering=False, debug=False)
    f32 = mybir.dt.float32
    bf16 = mybir.dt.bfloat16
    logits = nc.dram_tensor("logits", [R, N], bf16, kind="ExternalInput")
    targets = nc.dram_tensor("targets", [R, N], bf16, kind="ExternalInput")
    out = nc.dram_tensor("partials", [P, NT * SLOTS], f32,
                         kind="ExternalOutput")
    Lr = logits.ap().rearrange("(t p) n -> t p n", p=P)
    Tr = targets.ap().rearrange("(t p) n -> t p n", p=P)

    AF = mybir.ActivationFunctionType
    OP = mybir.AluOpType
    NMM = N // MMW

    with tile.TileContext(nc) as tc:
        with (
            tc.tile_pool(name="pL", bufs=4) as pL,
            tc.tile_pool(name="pT", bufs=4) as pT,
            tc.tile_pool(name="pLT", bufs=2) as pLT,
            tc.tile_pool(name="pV", bufs=2) as pV,
            tc.tile_pool(name="pScr", bufs=1) as pScr,
            tc.tile_pool(name="cnd", bufs=2) as cnd,
            tc.tile_pool(name="small", bufs=2) as small,
            tc.tile_pool(name="one", bufs=1) as one,
            tc.tile_pool(name="pJ", bufs=1) as pJ,
            tc.tile_pool(name="outp", bufs=1) as outp,
            tc.tile_pool(name="psum", bufs=1, space="PSUM") as psum,
        ):
            OUT = outp.tile([P, NT * SLOTS], f32)
            nc.gpsimd.memset(OUT, 0.0)
            ones = one.tile([P, 1], bf16)
            nc.gpsimd.memset(ones, 1.0)
            ltacc = psum.tile([1, MMW], f32)
            junk = pJ.tile([P, N], mybir.dt.float8e4)
            pend = None   # (Vt, LTt, mall, tau, s0) of the previous tile

            def emit_pend(Vt, LTt, cand, mall, tau, s0):
                # masked sum: MLT = (V >= tau) * LT, full row (result
                # discarded to a junk tile; only the accum matters)
                nc.vector.scalar_tensor_tensor(
                    out=junk, in0=Vt, scalar=tau, in1=LTt,
                    op0=OP.is_ge, op1=OP.mult,
                    accum_out=OUT[:, s0 + 2:s0 + 3])
                # sum_top softplus from the top-20 exp values: ln(V + 1)
                x20 = small.tile([P, 20], bf16, tag="x20")
                nc.scalar.activation(x20, mall[:, 0:20], AF.Ln,
                                     bias=1.0, scale=1.0,
                                     accum_out=OUT[:, s0 + 3:s0 + 4])
                # exactness channels on ACT (DVE copies stall against
                # concurrent GPSIMD SBUF traffic). cand is dumped after
                # match_replace: -1 entries mean "was in the top-16",
                # which the host flags conservatively.
                nc.scalar.activation(OUT[:, s0 + 8:s0 + 8 + CCH * 8],
                                     cand, AF.Copy)
                nc.scalar.activation(OUT[:, s0 + 72:s0 + 96],
                                     mall, AF.Copy)

            for t in range(NT):
                s0 = t * SLOTS
                LTt = pLT.tile([P, N], bf16, tag="LT")
                Vt = pV.tile([P, N], bf16, tag="V")
                for h in range(2):
                    sl = slice(h * H, (h + 1) * H)
                    Lh = pL.tile([P, H], bf16, tag="L")
                    nc.sync.dma_start(Lh, Lr[t][:, sl])
                    Th = pT.tile([P, H], bf16, tag="T")
                    nc.sync.dma_start(Th, Tr[t][:, sl])
                    eng = nc.gpsimd if h == 0 else nc.vector
                    eng.tensor_tensor(
                        out=LTt[:, sl], in0=Lh, in1=Th, op=OP.mult)
                    # V = exp(l); softplus accum via Ln(V + 1)
                    nc.scalar.activation(Vt[:, sl], Lh, AF.Exp)
                    scr = pScr.tile([P, H], bf16, tag="scr")
                    nc.scalar.activation(scr, Vt[:, sl], AF.Ln,
                                         bias=1.0, scale=1.0,
                                         accum_out=OUT[:, s0 + h:s0 + h + 1])
                    # sum_all(l*t): TensorE partition-sum of LT chunks
                    # into the persistent PSUM accumulator
                    for c in range(NMM // 2):
                        c0 = h * H + c * MMW
                        nc.tensor.matmul(
                            ltacc, ones, LTt[:, c0:c0 + MMW],
                            start=(t == 0 and h == 0 and c == 0),
                            stop=(t == NT - 1 and h == 1
                                  and c == NMM // 2 - 1))

                # top-20 in exp domain: per-chunk top-8, then cascade
                cand = cnd.tile([P, CCH * 8], bf16, tag="cand")
                for c in range(CCH):
                    nc.vector.max(out=cand[:, c * 8:(c + 1) * 8],
                                  in_=Vt[:, c * W:(c + 1) * W])
                # deferred masked sum + top-20 softplus of the previous
                # tile, emitted after max8 so the mask STT does not
                # overlap GPSIMD's products (SBUF port interference)
                if pend is not None:
                    emit_pend(*pend)
                mall = small.tile([P, 24], bf16, tag="mall")
                nc.vector.max(out=mall[:, 0:8], in_=cand)
                nc.vector.match_replace(out=cand, in_to_replace=mall[:, 0:8],
                                        in_values=cand, imm_value=-1.0)
                nc.vector.max(out=mall[:, 8:16], in_=cand)
                nc.vector.match_replace(out=cand, in_to_replace=mall[:, 8:16],
                                        in_values=cand, imm_value=-1.0)
                nc.vector.max(out=mall[:, 16:24], in_=cand)
                tau = mall[:, 19:20]   # 20th largest V; mall[:, 20] = 21st

                pend = (Vt, LTt, cand, mall, tau, s0)

            emit_pend(*pend)
            # read out the PSUM l*t total (one scalar on partition 0)
            scr5 = small.tile([1, MMW], f32, tag="psread")
            nc.scalar.activation(scr5, ltacc, AF.Copy,
                                 accum_out=OUT[0:1, 4:5])
            nc.sync.dma_start(out.ap(), OUT)

    # Force every activation onto one table (Exp+Ln+Copy live together
    # in natural_log_exp_and_others) so the engine never reloads tables.
    tabs = get_activation_tables(nc.m.arch)
    saved = {k: set(v) for k, v in tabs.items()}
    try:
        for k in tabs:
            if k != ACT_TABLE:
                tabs[k] = set()
        nc.compile()
    finally:
        for k, v in saved.items():
            tabs[k] = v
    return nc


def _get_program():
    global _PROGRAM
    if _PROGRAM is None:
        _PROGRAM = _build_program()
    return _PROGRAM


def _run_on_cores(logits, targets, trace=False, **kw):
    from concourse import bass_utils
    nc = _get_program()
    bf = ml_dtypes.bfloat16
    Lb = np.asarray(logits, dtype=np.float32).astype(bf)
    Tb = np.asarray(targets, dtype=np.float32).astype(bf)
    in_maps = [
        {"logits": np.ascontiguousarray(Lb[c * R:(c + 1) * R]),
         "targets": np.ascontiguousarray(Tb[c * R:(c + 1) * R])}
        for c in range(NCORES)
    ]
    return bass_utils.run_bass_kernel_spmd(
        nc, in_maps, core_ids=list(range(NCORES)), trace=trace, **kw)


def _host_fix_rows(logits, targets, rows):
    """Exact per-row recompute of the top-20 term, replicating the
    reference's tie-breaking (top_k on f32 sigmoid, stable by index)."""
    out = {}
    for r in rows:
        l = logits[r].astype(np.float32)
        t = targets[r].astype(np.float64)
        p = (1.0 / (1.0 + np.exp(-l.astype(np.float64)))).astype(np.float32)
        idx = np.argsort(-p, kind="stable")[:K]
        ld = l[idx].astype(np.float64)
        td = t[idx]
        sp = np.maximum(ld, 0) + np.log1p(np.exp(-np.abs(ld)))
        out[r] = float(np.sum(sp - ld * td))
    return out


def kernel(logits, targets, BCE_L):
    logits = np.asarray(logits, dtype=np.float32)
    targets = np.asarray(targets, dtype=np.float32)
    res = _run_on_cores(logits, targets)
    # partials[core]: [P, NT*SLOTS]; global row = core*R + t*P + p
    # slots: 0-1 sum softplus halves; 2 masked l*t; 3 sum_top softplus;
    #        8..72 candidate dump; 72..96 top-24 dump (exp domain);
    #        tile0 slot 4 partition 0 = core-wide sum l*t (from PSUM)
    bce_sum = 0.0
    me = np.zeros((NCORES, NT, P), dtype=np.float64)
    flag = np.zeros((NCORES, NT, P), dtype=bool)
    for c in range(NCORES):
        par = res.results[c]["partials"].astype(np.float64)
        bce_sum -= float(par[0, 4])
        for t in range(NT):
            s0 = t * SLOTS
            bce_sum += float(np.sum(par[:, s0:s0 + 2]))
            me[c, t] = par[:, s0 + 3] - par[:, s0 + 2]
            tau = par[:, s0 + 72 + 19]
            tau2 = par[:, s0 + 72 + 20]
            ch8 = par[:, s0 + 8:s0 + 8 + CCH * 8].reshape(P, CCH, 8)
            ch8 = ch8[:, :, 7]
            replaced = (ch8 == -1.0).any(axis=1)
            flag[c, t] = (ch8.max(axis=1) >= tau) | replaced | (tau2 == tau)
    me_rows = me.reshape(-1)
    bad = np.nonzero(flag.reshape(-1))[0]
    if bad.size:
        fixes = _host_fix_rows(logits, targets, bad.tolist())
        for r, v in fixes.items():
            me_rows[r] = v
    out = bce_sum / (B * N) + float(BCE_L[0]) * float(me_rows.sum()) / (B * K)
    return np.array(out, dtype=np.float32)


# revision 13
# speedup vs baseline: 1.0597x; 1.0207x over previous
"""Trainium2 Bass kernel for CustomBCEWithLogitsLoss (topk masking).

Math: with e = softplus(l) - l*t (elementwise BCE-with-logits),
  out = mean_all(e) + BCE_L * mean_{top20-by-logit per row}(e)
since top-k of sigmoid(logits) = top-k of logits, and the reference's
top-k BCE term equals e at those positions (-100 clamps never bind).

Strategy (vs a straight f32 port):
  * Host casts both inputs to bf16 - halves HBM traffic (20.5MB/core).
  * The whole top-k runs in the EXP DOMAIN: V = exp(l) (bf16, computed
    by ACT for softplus anyway) is monotone in l, so per-chunk max8,
    the top-20 cascade, tau, and the (V >= tau) mask all use V. L is
    freed right after Exp + product.
  * GPSIMD computes LT = l*t (bf16) - the only bulk GPSIMD work.
  * TensorE (otherwise idle) computes sum_all(l*t) for the BCE term:
    ones[128,1]^T @ LT chunks accumulated into one PSUM bank across
    all tiles (partition-dim reduction), one ACT read at the end.
    This removes every row-sum accumulation pass for l*t.
  * DVE: 8x max8 over 1250-col chunks, top-24 cascade, and one
    full-row masked-accum STT per tile: MLT = (V >= tau)*LT in place.

Per-core engine budget (us): DVE ~100, GPSIMD ~80, ACT ~74, PE ~25,
DMA ~62.

Exactness: selection matches the reference's f32 top-20 exactly unless
bf16 quantization ties the 20/21 boundary (tau2 == tau in exact bf16
bits) or a chunk's top-8 may have missed a top-20 value (ch8 >= tau).
Flagged rows (~37%: at rank 20 of 10000 N(0,1) values the order-stat
gap matches the bf16 quantum) are recomputed exactly on host from the
original f32 inputs. Host combines partials in f64.
"""

import numpy as np
import ml_dtypes

B, N, K = 4096, 10000, 20
NCORES = 8
R = B // NCORES          # rows per core
P = 128                  # partitions
NT = R // P              # tiles per core
H = N // 2               # half-row width
CCH = 8                  # candidate chunks per row
W = N // CCH             # candidate chunk width (1250)
MMW = 500                # matmul moving chunk width (PSUM bank fit)
SLOTS = 96               # per-tile output slots
ACT_TABLE = "natural_log_exp_and_others"

_PROGRAM = None


def _build_program():
    import concourse.bacc as bacc
    import concourse.tile as tile
    import concourse.mybir as mybir
    from concourse.hw_specs import get_activation_tables

    nc = bacc.Bacc("TRN2", target_bir_lowering=False, debug=False)
    f32 = mybir.dt.float32
    bf16 = mybir.dt.bfloat16
    logits = nc.dram_tensor("logits", [R, N], bf16, kind="ExternalInput")
    targets = nc.dram_tensor("targets", [R, N], bf16, kind="ExternalInput")
    out = nc.dram_tensor("partials", [P, NT * SLOTS], f32,
                         kind="ExternalOutput")
    Lr = logits.ap().rearrange("(t p) n -> t p n", p=P)
    Tr = targets.ap().rearrange("(t p) n -> t p n", p=P)

    AF = mybir.ActivationFunctionType
    OP = mybir.AluOpType
    NMM = N // MMW

    with tile.TileContext(nc) as tc:
        with (
            tc.tile_pool(name="pL", bufs=4) as pL,
            tc.tile_pool(name="pT", bufs=4) as pT,
            tc.tile_pool(name="pLT", bufs=2) as pLT,
            tc.tile_pool(name="pV", bufs=2) as pV,
            tc.tile_pool(name="pScr", bufs=1) as pScr,
            tc.tile_pool(name="cnd", bufs=2) as cnd,
            tc.tile_pool(name="small", bufs=2) as small,
            tc.tile_pool(name="one", bufs=1) as one,
            tc.tile_pool(name="pJ", bufs=1) as pJ,
            tc.tile_pool(name="outp", bufs=1) as outp,
            tc.tile_pool(name="psum", bufs=1, space="PSUM") as psum,
        ):
            OUT = outp.tile([P, NT * SLOTS], f32)
            nc.gpsimd.memset(OUT, 0.0)
            ones = one.tile([P, 1], bf16)
            nc.gpsimd.memset(ones, 1.0)
            ltacc = psum.tile([1, MMW], f32)
            junk = pJ.tile([P, N], mybir.dt.float8e4)
            pend = None   # (Vt, LTt, mall, tau, s0) of the previous tile

            def emit_pend(Vt, LTt, cand, mall, tau, s0):
                # masked sum: MLT = (V >= tau) * LT, full row (result
                # discarded to a junk tile; only the accum matters)
                nc.vector.scalar_tensor_tensor(
                    out=junk, in0=Vt, scalar=tau, in1=LTt,
                    op0=OP.is_ge, op1=OP.mult,
                    accum_out=OUT[:, s0 + 2:s0 + 3])
                # sum_top softplus from the top-20 exp values: ln(V + 1)
                x20 = small.tile([P, 20], bf16, tag="x20")
                nc.scalar.activation(x20, mall[:, 0:20], AF.Ln,
                                     bias=1.0, scale=1.0,
                                     accum_out=OUT[:, s0 + 3:s0 + 4])
                # exactness channels on ACT (DVE copies stall against
                # concurrent GPSIMD SBUF traffic). cand is dumped after
                # match_replace: -1 entries mean "was in the top-16",
                # which the host flags conservatively.
                nc.scalar.activation(OUT[:, s0 + 8:s0 + 8 + CCH * 8],
                                     cand, AF.Copy)
                nc.scalar.activation(OUT[:, s0 + 72:s0 + 96],
                                     mall, AF.Copy)

            for t in range(NT):
                s0 = t * SLOTS
                LTt = pLT.tile([P, N], bf16, tag="LT")
                Vt = pV.tile([P, N], bf16, tag="V")
                for h in range(2):
                    sl = slice(h * H, (h + 1) * H)
                    Lh = pL.tile([P, H], bf16, tag="L")
                    nc.sync.dma_start(Lh, Lr[t][:, sl])
                    Th = pT.tile([P, H], bf16, tag="T")
                    nc.sync.dma_start(Th, Tr[t][:, sl])
                    # both halves on DVE TT (2x bf16): GPSIMD tensor
                    # ops contend with DVE on SBUF ports (~2x mutual
                    # slowdown when overlapped), so keep GPSIMD idle
                    nc.vector.tensor_tensor(
                        out=LTt[:, sl], in0=Lh, in1=Th, op=OP.mult)
                    # V = exp(l); softplus accum via Ln(V + 1)
                    nc.scalar.activation(Vt[:, sl], Lh, AF.Exp)
                    scr = pScr.tile([P, H], bf16, tag="scr")
                    nc.scalar.activation(scr, Vt[:, sl], AF.Ln,
                                         bias=1.0, scale=1.0,
                                         accum_out=OUT[:, s0 + h:s0 + h + 1])
                    # sum_all(l*t): TensorE partition-sum of LT chunks
                    # into the persistent PSUM accumulator
                    for c in range(NMM // 2):
                        c0 = h * H + c * MMW
                        nc.tensor.matmul(
                            ltacc, ones, LTt[:, c0:c0 + MMW],
                            start=(t == 0 and h == 0 and c == 0),
                            stop=(t == NT - 1 and h == 1
                                  and c == NMM // 2 - 1))

                # top-20 in exp domain: per-chunk top-8, then cascade
                cand = cnd.tile([P, CCH * 8], bf16, tag="cand")
                for c in range(CCH):
                    nc.vector.max(out=cand[:, c * 8:(c + 1) * 8],
                                  in_=Vt[:, c * W:(c + 1) * W])
                # deferred masked sum + top-20 softplus of the previous
                # tile, emitted after max8 so the mask STT does not
                # overlap GPSIMD's products (SBUF port interference)
                if pend is not None:
                    emit_pend(*pend)
                mall = small.tile([P, 24], bf16, tag="mall")
                nc.vector.max(out=mall[:, 0:8], in_=cand)
                nc.vector.match_replace(out=cand, in_to_replace=mall[:, 0:8],
                                        in_values=cand, imm_value=-1.0)
                nc.vector.max(out=mall[:, 8:16], in_=cand)
                nc.vector.match_replace(out=cand, in_to_replace=mall[:, 8:16],
                                        in_values=cand, imm_value=-1.0)
                nc.vector.max(out=mall[:, 16:24], in_=cand)
                tau = mall[:, 19:20]   # 20th largest V; mall[:, 20] = 21st

                pend = (Vt, LTt, cand, mall, tau, s0)

            emit_pend(*pend)
            # read out the PSUM l*t total (one scalar on partition 0)
            scr5 = small.tile([1, MMW], f32, tag="psread")
            nc.scalar.activation(scr5, ltacc, AF.Copy,
                                 accum_out=OUT[0:1, 4:5])
            nc.sync.dma_start(out.ap(), OUT)

    # Force every activation onto one table (Exp+Ln+Copy live together
    # in natural_log_exp_and_others) so the engine never reloads tables.
    tabs = get_activation_tables(nc.m.arch)
    saved = {k: set(v) for k, v in tabs.items()}
    try:
        for k in tabs:
            if k != ACT_TABLE:
                tabs[k] = set()
        nc.compile()
    finally:
        for k, v in saved.items():
            tabs[k] = v
    return nc


def _get_program():
    global _PROGRAM
    if _PROGRAM is None:
        _PROGRAM = _build_program()
    return _PROGRAM


def _run_on_cores(logits, targets, trace=False, **kw):
    from concourse import bass_utils
    nc = _get_program()
    bf = ml_dtypes.bfloat16
    Lb = np.asarray(logits, dtype=np.float32).astype(bf)
    Tb = np.asarray(targets, dtype=np.float32).astype(bf)
    in_maps = [
        {"logits": np.ascontiguousarray(Lb[c * R:(c + 1) * R]),
         "targets": np.ascontiguousarray(Tb[c * R:(c + 1) * R])}
        for c in range(NCORES)
    ]
    return bass_utils.run_bass_kernel_spmd(
        nc, in_maps, core_ids=list(range(NCORES)), trace=trace, **kw)


def _host_fix_rows(logits, targets, rows):
    """Exact per-row recompute of the top-20 term, replicating the
    reference's tie-breaking (top_k on f32 sigmoid, stable by index)."""
    out = {}
    for r in rows:
        l = logits[r].astype(np.float32)
        t = targets[r].astype(np.float64)
        p = (1.0 / (1.0 + np.exp(-l.astype(np.float64)))).astype(np.float32)
        idx = np.argsort(-p, kind="stable")[:K]
        ld = l[idx].astype(np.float64)
        td = t[idx]
        sp = np.maximum(ld, 0) + np.log1p(np.exp(-np.abs(ld)))
        out[r] = float(np.sum(sp - ld * td))
    return out


def kernel(logits, targets, BCE_L):
    logits = np.asarray(logits, dtype=np.float32)
    targets = np.asarray(targets, dtype=np.float32)
    res = _run_on_cores(logits, targets)
    # partials[core]: [P, NT*SLOTS]; global row = core*R + t*P + p
    # slots: 0-1 sum softplus halves; 2 masked l*t; 3 sum_top softplus;
    #        8..72 candidate dump; 72..96 top-24 dump (exp domain);
    #        tile0 slot 4 partition 0 = core-wide sum l*t (from PSUM)
    bce_sum = 0.0
    me = np.zeros((NCORES, NT, P), dtype=np.float64)
    flag = np.zeros((NCORES, NT, P), dtype=bool)
    for c in range(NCORES):
        par = res.results[c]["partials"].astype(np.float64)
        bce_sum -= float(par[0, 4])
        for t in range(NT):
            s0 = t * SLOTS
            bce_sum += float(np.sum(par[:, s0:s0 + 2]))
            me[c, t] = par[:, s0 + 3] - par[:, s0 + 2]
            tau = par[:, s0 + 72 + 19]
            tau2 = par[:, s0 + 72 + 20]
            ch8 = par[:, s0 + 8:s0 + 8 + CCH * 8].reshape(P, CCH, 8)
            ch8 = ch8[:, :, 7]
            replaced = (ch8 == -1.0).any(axis=1)
            flag[c, t] = (ch8.max(axis=1) >= tau) | replaced | (tau2 == tau)
    me_rows = me.reshape(-1)
    bad = np.nonzero(flag.reshape(-1))[0]
    if bad.size:
        fixes = _host_fix_rows(logits, targets, bad.tolist())
        for r, v in fixes.items():
            me_rows[r] = v
    out = bce_sum / (B * N) + float(BCE_L[0]) * float(me_rows.sum()) / (B * K)
    return np.array(out, dtype=np.float32)
